# revision 1
# baseline (speedup 1.0000x reference)
"""Trainium2 Bass kernel for NeighborMLPConvLayerLinear (gnn_message_passing).

Strategy (8 NeuronCores, SPMD):
  - Edges (E=1.6M) are sharded by output segment: core c owns segments
    [c*6250, (c+1)*6250) = 200k edges. Segments are uniform (row_splits =
    arange*32), so segment reduction is a fixed stride-32 sum.
  - Gather: a bf16 table row per input point i: [x_in[i] (32) | in_features[i]
    (32) | 64 zeros] = 128 bf16 = 256B. dma_gather (SBUF-source,
    transpose=True) delivers gathered rows channels-on-partitions, which feeds
    the MLP matmuls directly. dma_gather indices are int16 (max 32767) so the
    50000-row table is split in two halves (lo: rows 0..24999, hi: rows
    25000..49999); each edge is gathered from both streams, with the
    stream not containing its row pointing at a dedicated all-zero row 0.
    Merged rows = lo + hi. Gathered zeros multiply in_features=0 so padded
    slots contribute exactly 0 to segment sums.
  - MLP: h = gelu(W1a^T x_g + W1b^T x_out[seg] + b1) on PE+ACT
    (channels-on-partitions, x_out broadcast along the 32-slot segment via a
    stride-0 access pattern); edge_out = (W2^T h + b2)/32 * F_g; segment sums
    via tensor_reduce over the innermost 32-slot axis.
"""
import sys

sys.path.insert(0, "/opt/trn_rl_repo")

import numpy as np
import ml_dtypes

from concourse import bacc, bass, mybir, tile
from concourse import bass_utils

BF16 = mybir.dt.bfloat16
F32 = mybir.dt.float32
I16 = mybir.dt.int16

N = 50000
M = 50000
DEG = 32
C_IN = 32
HID = 64
C_OUT = 32

NCORES = 8
SEG_PER_CORE = M // NCORES            # 6250
E_PER_CORE = SEG_PER_CORE * DEG       # 200000
SLOTS = 204800                        # padded to a multiple of CH
CH = 4096                             # gather-chunk (slots per dma_gather)
NCHUNK = SLOTS // CH                  # 50
SEG_PAD = SLOTS // DEG                # 6400 segments incl. padding
SEG_PER_CHUNK = CH // DEG             # 128
PSUM_CH = 1024                        # edges per psum tile
KSUB = CH // PSUM_CH                  # 4

HALF = 25000                          # rows per table half
RANKS = 196                           # ceil(25001/128): row r at partition r%128, rank r//128
TOK = RANKS * 128                     # 25088 token slots per table

_NC_CACHE = {}


def build_nc():
    if "nc" in _NC_CACHE:
        return _NC_CACHE["nc"]
    nc = bacc.Bacc("TRN2", target_bir_lowering=False, debug=False,
                   num_devices=NCORES)

    tbl_lo = nc.dram_tensor("tbl_lo", [128, RANKS * 128], BF16, kind="ExternalInput").ap()
    tbl_hi = nc.dram_tensor("tbl_hi", [128, RANKS * 128], BF16, kind="ExternalInput").ap()
    idx_lo = nc.dram_tensor("idx_lo", [NCHUNK, 128, CH // 16], I16, kind="ExternalInput").ap()
    idx_hi = nc.dram_tensor("idx_hi", [NCHUNK, 128, CH // 16], I16, kind="ExternalInput").ap()
    xo = nc.dram_tensor("xo", [C_IN, SEG_PAD], BF16, kind="ExternalInput").ap()
    wx = nc.dram_tensor("wx", [64, HID], BF16, kind="ExternalInput").ap()
    w1b = nc.dram_tensor("w1b", [C_IN, HID], BF16, kind="ExternalInput").ap()
    w2 = nc.dram_tensor("w2", [HID + 1, C_OUT], BF16, kind="ExternalInput").ap()
    b1 = nc.dram_tensor("b1", [HID, 1], F32, kind="ExternalInput").ap()
    out = nc.dram_tensor("out", [C_OUT, SEG_PAD], F32, kind="ExternalOutput").ap()

    with tile.TileContext(nc) as tc:
        with (
            tc.tile_pool(name="tbl", bufs=1) as tblp,
            tc.tile_pool(name="w", bufs=1) as wp,
            tc.tile_pool(name="idx", bufs=2) as idxp,
            tc.tile_pool(name="g", bufs=2) as gp,
            tc.tile_pool(name="gm", bufs=2) as gmp,
            tc.tile_pool(name="h", bufs=1) as hp,
            tc.tile_pool(name="eo", bufs=2) as eop,
            tc.tile_pool(name="red", bufs=2) as redp,
            tc.tile_pool(name="ps1", bufs=2, space="PSUM") as ps1,
            tc.tile_pool(name="ps2", bufs=2, space="PSUM") as ps2,
        ):
            sb_lo = tblp.tile([128, RANKS * 128], BF16, tag="tlo")
            sb_hi = tblp.tile([128, RANKS * 128], BF16, tag="thi")
            nc.sync.dma_start(out=sb_lo[:], in_=tbl_lo[:])
            nc.sync.dma_start(out=sb_hi[:], in_=tbl_hi[:])

            sb_xo = wp.tile([C_IN, SEG_PAD], BF16, tag="xo")
            nc.sync.dma_start(out=sb_xo[:], in_=xo[:])
            sb_wx = wp.tile([64, HID], BF16, tag="wx")
            nc.sync.dma_start(out=sb_wx[:], in_=wx[:])
            sb_w1b = wp.tile([C_IN, HID], BF16, tag="w1b")
            nc.sync.dma_start(out=sb_w1b[:], in_=w1b[:])
            sb_w2 = wp.tile([HID + 1, C_OUT], BF16, tag="w2")
            nc.sync.dma_start(out=sb_w2[:], in_=w2[:])
            sb_b1 = wp.tile([HID, 1], F32, tag="b1")
            nc.sync.dma_start(out=sb_b1[:], in_=b1[:])

            # h staging: [HID+1, 2*PSUM_CH]; row HID stays 1.0 (bias-via-matmul)
            h_all = hp.tile([HID + 1, 2 * PSUM_CH], BF16, tag="h")
            nc.vector.memset(h_all[HID:HID + 1, :], 1.0)

            for t in range(NCHUNK):
                ilo = idxp.tile([128, CH // 16], I16, tag="ilo")
                nc.sync.dma_start(out=ilo[:], in_=idx_lo[t])
                ihi = idxp.tile([128, CH // 16], I16, tag="ihi")
                nc.sync.dma_start(out=ihi[:], in_=idx_hi[t])

                glo = gp.tile([128, CH], BF16, tag="glo")
                nc.gpsimd.dma_gather(
                    out_ap=glo[:].unsqueeze(1), in_ap=sb_lo[:], idxs_ap=ilo[:],
                    num_idxs=CH, num_idxs_reg=CH, elem_size=128, transpose=True,
                    sbuf_tokens_per_rank=128, sbuf_free_dim_per_rank=256, single_packet=False,
                )
                ghi = gp.tile([128, CH], BF16, tag="ghi")
                nc.gpsimd.dma_gather(
                    out_ap=ghi[:].unsqueeze(1), in_ap=sb_hi[:], idxs_ap=ihi[:],
                    num_idxs=CH, num_idxs_reg=CH, elem_size=128, transpose=True,
                    sbuf_tokens_per_rank=128, sbuf_free_dim_per_rank=256, single_packet=False,
                )
                # merged [x | F] channels
                gm = gmp.tile([64, CH], BF16, tag="gm")
                nc.vector.tensor_tensor(out=gm[:], in0=glo[0:64, :], in1=ghi[0:64, :],
                                        op=mybir.AluOpType.add)

                red = redp.tile([C_OUT, SEG_PER_CHUNK], F32, tag="red")
                for k in range(KSUB):
                    e0 = k * PSUM_CH
                    p1 = ps1.tile([HID, PSUM_CH], F32, tag="p1")
                    for j in range(PSUM_CH // 512):
                        c0 = e0 + j * 512
                        s0 = (t * CH + c0) // DEG  # first segment of this 512-block
                        nc.tensor.matmul(out=p1[:, j * 512:(j + 1) * 512],
                                         lhsT=sb_wx[:], rhs=gm[:, c0:c0 + 512],
                                         start=True, stop=False)
                        xo_b = sb_xo[:, s0:s0 + 16].unsqueeze(2).to_broadcast(
                            [C_IN, 16, DEG])
                        nc.tensor.matmul(out=p1[:, j * 512:(j + 1) * 512],
                                         lhsT=sb_w1b[:], rhs=xo_b,
                                         start=False, stop=True)
                    hs = h_all[:, (k % 2) * PSUM_CH:(k % 2 + 1) * PSUM_CH]
                    nc.scalar.activation(hs[0:HID, :], p1[:],
                                         mybir.ActivationFunctionType.Gelu,
                                         bias=sb_b1[:], scale=1.0)
                    p2 = ps2.tile([C_OUT, PSUM_CH], F32, tag="p2")
                    for j in range(PSUM_CH // 512):
                        nc.tensor.matmul(out=p2[:, j * 512:(j + 1) * 512],
                                         lhsT=sb_w2[:],
                                         rhs=hs[:, j * 512:(j + 1) * 512],
                                         start=True, stop=True)
                    eo = eop.tile([C_OUT, PSUM_CH], BF16, tag="eo")
                    nc.vector.tensor_tensor(out=eo[:], in0=p2[:],
                                            in1=gm[C_IN:64, e0:e0 + PSUM_CH],
                                            op=mybir.AluOpType.mult)
                    nc.vector.tensor_reduce(
                        out=red[:, k * (PSUM_CH // DEG):(k + 1) * (PSUM_CH // DEG)],
                        in_=eo[:].rearrange("p (s e) -> p s e", e=DEG),
                        axis=mybir.AxisListType.X, op=mybir.AluOpType.add)
                nc.sync.dma_start(
                    out=out[:, t * SEG_PER_CHUNK:(t + 1) * SEG_PER_CHUNK],
                    in_=red[:])
    nc.compile()
    _NC_CACHE["nc"] = nc
    return nc


def _prep_core(idx_core, x_out_core):
    """Per-core host prep: int16 gather streams + transposed x_out."""
    v = idx_core.astype(np.int64)
    lo = np.where(v < HALF, v + 1, 0).astype(np.int16)
    hi = np.where(v >= HALF, v - (HALF - 1), 0).astype(np.int16)
    pad = SLOTS - v.shape[0]
    lo = np.concatenate([lo, np.zeros(pad, np.int16)])
    hi = np.concatenate([hi, np.zeros(pad, np.int16)])

    def wrap(a):
        # slot i -> partition i%16, col i//16; replicated over 8 groups
        w = a.reshape(NCHUNK, CH // 16, 16).transpose(0, 2, 1)  # [NCHUNK,16,CH/16]
        return np.tile(w, (1, 8, 1)).copy()                     # [NCHUNK,128,CH/16]

    xo = np.zeros((C_IN, SEG_PAD), dtype=ml_dtypes.bfloat16)
    xo[:, :SEG_PER_CORE] = x_out_core.T.astype(ml_dtypes.bfloat16)
    return wrap(lo), wrap(hi), xo


def kernel(x_in, x_out, in_features, neighbors_index, neighbors_row_splits,
           W1, b1, W2, b2):
    x_in = np.asarray(x_in, np.float32)
    x_out = np.asarray(x_out, np.float32)
    in_features = np.asarray(in_features, np.float32)
    idx = np.asarray(neighbors_index, np.int32)
    W1 = np.asarray(W1, np.float32)
    b1v = np.asarray(b1, np.float32)
    W2 = np.asarray(W2, np.float32)
    b2v = np.asarray(b2, np.float32)

    # table rows: [x_in | in_features | zeros]; row 0 of each half = zeros
    rows = np.zeros((TOK, 128), dtype=ml_dtypes.bfloat16)
    rows[1:N // 2 + 1, 0:C_IN] = x_in[:HALF].astype(ml_dtypes.bfloat16)
    rows[1:N // 2 + 1, C_IN:64] = in_features[:HALF].astype(ml_dtypes.bfloat16)
    tbl_lo = rows.reshape(RANKS, 128, 128).transpose(1, 0, 2).reshape(128, RANKS * 128).copy()
    rows[:] = 0
    rows[1:N // 2 + 1, 0:C_IN] = x_in[HALF:].astype(ml_dtypes.bfloat16)
    rows[1:N // 2 + 1, C_IN:64] = in_features[HALF:].astype(ml_dtypes.bfloat16)
    tbl_hi = rows.reshape(RANKS, 128, 128).transpose(1, 0, 2).reshape(128, RANKS * 128).copy()

    wx = np.zeros((64, HID), dtype=ml_dtypes.bfloat16)
    wx[0:C_IN] = W1[0:C_IN].astype(ml_dtypes.bfloat16)
    w1b = W1[C_IN:].astype(ml_dtypes.bfloat16)
    w2aug = np.zeros((HID + 1, C_OUT), dtype=ml_dtypes.bfloat16)
    w2aug[0:HID] = (W2 / DEG).astype(ml_dtypes.bfloat16)
    w2aug[HID] = (b2v / DEG).astype(ml_dtypes.bfloat16)
    b1c = b1v.reshape(HID, 1).copy()

    in_maps = []
    for c in range(NCORES):
        ilo, ihi, xo = _prep_core(
            idx[c * E_PER_CORE:(c + 1) * E_PER_CORE],
            x_out[c * SEG_PER_CORE:(c + 1) * SEG_PER_CORE])
        in_maps.append({
            "tbl_lo": tbl_lo, "tbl_hi": tbl_hi,
            "idx_lo": ilo, "idx_hi": ihi, "xo": xo,
            "wx": wx, "w1b": w1b, "w2": w2aug, "b1": b1c,
        })

    global _LAST_IN_MAPS
    _LAST_IN_MAPS = in_maps
    nc = build_nc()
    res = bass_utils.run_bass_kernel_spmd(nc, in_maps, list(range(NCORES))).results
    out = np.empty((M, C_OUT), np.float32)
    for c in range(NCORES):
        out[c * SEG_PER_CORE:(c + 1) * SEG_PER_CORE] = \
            res[c]["out"][:, :SEG_PER_CORE].T
    return out



# revision 3
# speedup vs baseline: 8.4429x; 8.4429x over previous
"""Trainium2 Bass kernel for NeighborMLPConvLayerLinear (gnn_message_passing).

Strategy (8 NeuronCores, SPMD):
  - Edges (E=1.6M) are sharded by output segment: core c owns segments
    [c*6250, (c+1)*6250) = 200k edges. Segments are uniform (row_splits =
    arange*32), so segment reduction is a fixed stride-32 sum.
  - Gather: a bf16 table row per input point i: [x_in[i] (32) | in_features[i]
    (32) | 64 zeros] = 128 bf16 = 256B. dma_gather (SBUF-source,
    transpose=True) delivers gathered rows channels-on-partitions, which feeds
    the MLP matmuls directly. dma_gather indices are int16 (max 32767) so the
    50000-row table is split in two halves (lo: rows 0..24999, hi: rows
    25000..49999); each edge is gathered from both streams, with the
    stream not containing its row pointing at a dedicated all-zero row 0.
    Merged rows = lo + hi. Gathered zeros multiply in_features=0 so padded
    slots contribute exactly 0 to segment sums.
  - MLP: h = gelu(W1a^T x_g + W1b^T x_out[seg] + b1) on PE+ACT
    (channels-on-partitions, x_out broadcast along the 32-slot segment via a
    stride-0 access pattern); edge_out = (W2^T h + b2)/32 * F_g; segment sums
    via tensor_reduce over the innermost 32-slot axis.
"""
import sys

sys.path.insert(0, "/opt/trn_rl_repo")

import numpy as np
import ml_dtypes

from concourse import bacc, bass, mybir, tile
from concourse import bass_utils

BF16 = mybir.dt.bfloat16
F32 = mybir.dt.float32
I16 = mybir.dt.int16

N = 50000
M = 50000
DEG = 32
C_IN = 32
HID = 64
C_OUT = 32

NCORES = 8
SEG_PER_CORE = M // NCORES            # 6250
E_PER_CORE = SEG_PER_CORE * DEG       # 200000
SLOTS = 204800                        # padded to a multiple of CH
CH = 4096                             # gather-chunk (slots per dma_gather)
NCHUNK = SLOTS // CH                  # 50
SEG_PAD = SLOTS // DEG                # 6400 segments incl. padding
SEG_PER_CHUNK = CH // DEG             # 128
PSUM_CH = 1024                        # edges per psum tile
KSUB = CH // PSUM_CH                  # 4

HALF = 25000                          # rows per table half
RANKS = 196                           # ceil(25001/128): row r at partition r%128, rank r//128
TOK = RANKS * 128                     # 25088 token slots per table

_NC_CACHE = {}


def build_nc():
    if "nc" in _NC_CACHE:
        return _NC_CACHE["nc"]
    nc = bacc.Bacc("TRN2", target_bir_lowering=False, debug=False,
                   num_devices=NCORES)

    tbl_lo = nc.dram_tensor("tbl_lo", [128, RANKS * 128], BF16, kind="ExternalInput").ap()
    tbl_hi = nc.dram_tensor("tbl_hi", [128, RANKS * 128], BF16, kind="ExternalInput").ap()
    idx_lo = nc.dram_tensor("idx_lo", [NCHUNK, 128, CH // 16], I16, kind="ExternalInput").ap()
    idx_hi = nc.dram_tensor("idx_hi", [NCHUNK, 128, CH // 16], I16, kind="ExternalInput").ap()
    xo = nc.dram_tensor("xo", [C_IN, SEG_PAD], BF16, kind="ExternalInput").ap()
    wx = nc.dram_tensor("wx", [64, HID], BF16, kind="ExternalInput").ap()
    w1b = nc.dram_tensor("w1b", [C_IN, HID], BF16, kind="ExternalInput").ap()
    w2 = nc.dram_tensor("w2", [HID + 1, C_OUT], BF16, kind="ExternalInput").ap()
    b1 = nc.dram_tensor("b1", [HID, 1], F32, kind="ExternalInput").ap()
    out = nc.dram_tensor("out", [C_OUT, SEG_PAD], F32, kind="ExternalOutput").ap()

    with tile.TileContext(nc) as tc:
        with (
            tc.tile_pool(name="tbl", bufs=1) as tblp,
            tc.tile_pool(name="w", bufs=1) as wp,
            tc.tile_pool(name="idx", bufs=2) as idxp,
            tc.tile_pool(name="g", bufs=2) as gp,
            tc.tile_pool(name="gm", bufs=2) as gmp,
            tc.tile_pool(name="h", bufs=1) as hp,
            tc.tile_pool(name="eo", bufs=2) as eop,
            tc.tile_pool(name="red", bufs=2) as redp,
            tc.tile_pool(name="ps1", bufs=2, space="PSUM") as ps1,
            tc.tile_pool(name="ps2", bufs=2, space="PSUM") as ps2,
        ):
            sb_lo = tblp.tile([128, RANKS * 128], BF16, tag="tlo")
            sb_hi = tblp.tile([128, RANKS * 128], BF16, tag="thi")
            nc.sync.dma_start(out=sb_lo[:], in_=tbl_lo[:])
            nc.sync.dma_start(out=sb_hi[:], in_=tbl_hi[:])

            sb_xo = wp.tile([C_IN, SEG_PAD], BF16, tag="xo")
            nc.sync.dma_start(out=sb_xo[:], in_=xo[:])
            sb_wx = wp.tile([64, HID], BF16, tag="wx")
            nc.sync.dma_start(out=sb_wx[:], in_=wx[:])
            sb_w1b = wp.tile([C_IN, HID], BF16, tag="w1b")
            nc.sync.dma_start(out=sb_w1b[:], in_=w1b[:])
            sb_w2 = wp.tile([HID + 1, C_OUT], BF16, tag="w2")
            nc.sync.dma_start(out=sb_w2[:], in_=w2[:])
            sb_b1 = wp.tile([HID, 1], F32, tag="b1")
            nc.sync.dma_start(out=sb_b1[:], in_=b1[:])

            # h staging: [HID+1, 2*PSUM_CH]; row HID stays 1.0 (bias-via-matmul)
            h_all = hp.tile([HID + 1, 2 * PSUM_CH], BF16, tag="h")
            nc.vector.memset(h_all[HID:HID + 1, :], 1.0)

            for t in range(NCHUNK):
                ilo = idxp.tile([128, CH // 16], I16, tag="ilo")
                nc.sync.dma_start(out=ilo[:], in_=idx_lo[t])
                ihi = idxp.tile([128, CH // 16], I16, tag="ihi")
                nc.sync.dma_start(out=ihi[:], in_=idx_hi[t])

                glo = gp.tile([128, CH], BF16, tag="glo")
                nc.gpsimd.dma_gather(
                    out_ap=glo[:].unsqueeze(1), in_ap=sb_lo[:], idxs_ap=ilo[:],
                    num_idxs=CH, num_idxs_reg=CH, elem_size=128, transpose=True,
                    sbuf_tokens_per_rank=128, sbuf_free_dim_per_rank=256, single_packet=False,
                )
                ghi = gp.tile([128, CH], BF16, tag="ghi")
                nc.gpsimd.dma_gather(
                    out_ap=ghi[:].unsqueeze(1), in_ap=sb_hi[:], idxs_ap=ihi[:],
                    num_idxs=CH, num_idxs_reg=CH, elem_size=128, transpose=True,
                    sbuf_tokens_per_rank=128, sbuf_free_dim_per_rank=256, single_packet=False,
                )
                # merged [x | F] channels
                gm = gmp.tile([64, CH], BF16, tag="gm")
                nc.vector.tensor_tensor(out=gm[:], in0=glo[0:64, :], in1=ghi[0:64, :],
                                        op=mybir.AluOpType.add)

                red = redp.tile([C_OUT, SEG_PER_CHUNK], F32, tag="red")
                for k in range(KSUB):
                    e0 = k * PSUM_CH
                    p1 = ps1.tile([HID, PSUM_CH], F32, tag="p1")
                    for j in range(PSUM_CH // 512):
                        c0 = e0 + j * 512
                        s0 = (t * CH + c0) // DEG  # first segment of this 512-block
                        nc.tensor.matmul(out=p1[:, j * 512:(j + 1) * 512],
                                         lhsT=sb_wx[:], rhs=gm[:, c0:c0 + 512],
                                         start=True, stop=False)
                        xo_b = sb_xo[:, s0:s0 + 16].unsqueeze(2).to_broadcast(
                            [C_IN, 16, DEG])
                        nc.tensor.matmul(out=p1[:, j * 512:(j + 1) * 512],
                                         lhsT=sb_w1b[:], rhs=xo_b,
                                         start=False, stop=True)
                    hs = h_all[:, (k % 2) * PSUM_CH:(k % 2 + 1) * PSUM_CH]
                    nc.scalar.activation(hs[0:HID, :], p1[:],
                                         mybir.ActivationFunctionType.Gelu,
                                         bias=sb_b1[:], scale=1.0)
                    p2 = ps2.tile([C_OUT, PSUM_CH], F32, tag="p2")
                    for j in range(PSUM_CH // 512):
                        nc.tensor.matmul(out=p2[:, j * 512:(j + 1) * 512],
                                         lhsT=sb_w2[:],
                                         rhs=hs[:, j * 512:(j + 1) * 512],
                                         start=True, stop=True)
                    eo = eop.tile([C_OUT, PSUM_CH], BF16, tag="eo")
                    nc.vector.tensor_tensor(out=eo[:], in0=p2[:],
                                            in1=gm[C_IN:64, e0:e0 + PSUM_CH],
                                            op=mybir.AluOpType.mult)
                    nc.vector.tensor_reduce(
                        out=red[:, k * (PSUM_CH // DEG):(k + 1) * (PSUM_CH // DEG)],
                        in_=eo[:].rearrange("p (s e) -> p s e", e=DEG),
                        axis=mybir.AxisListType.X, op=mybir.AluOpType.add)
                nc.sync.dma_start(
                    out=out[:, t * SEG_PER_CHUNK:(t + 1) * SEG_PER_CHUNK],
                    in_=red[:])
    nc.compile()
    _NC_CACHE["nc"] = nc
    return nc


def _prep_core(idx_core, x_out_core):
    """Per-core host prep: int16 gather streams + transposed x_out."""
    v = idx_core.astype(np.int64)
    lo = np.where(v < HALF, v + 1, 0).astype(np.int16)
    hi = np.where(v >= HALF, v - (HALF - 1), 0).astype(np.int16)
    pad = SLOTS - v.shape[0]
    lo = np.concatenate([lo, np.zeros(pad, np.int16)])
    hi = np.concatenate([hi, np.zeros(pad, np.int16)])

    def wrap(a):
        # slot i -> partition i%16, col i//16; replicated over 8 groups
        w = a.reshape(NCHUNK, CH // 16, 16).transpose(0, 2, 1)  # [NCHUNK,16,CH/16]
        return np.tile(w, (1, 8, 1)).copy()                     # [NCHUNK,128,CH/16]

    xo = np.zeros((C_IN, SEG_PAD), dtype=ml_dtypes.bfloat16)
    xo[:, :SEG_PER_CORE] = x_out_core.T.astype(ml_dtypes.bfloat16)
    return wrap(lo), wrap(hi), xo


def prep_in_maps(x_in, x_out, in_features, neighbors_index, neighbors_row_splits,
                 W1, b1, W2, b2):
    x_in = np.asarray(x_in, np.float32)
    x_out = np.asarray(x_out, np.float32)
    in_features = np.asarray(in_features, np.float32)
    idx = np.asarray(neighbors_index, np.int32)
    W1 = np.asarray(W1, np.float32)
    b1v = np.asarray(b1, np.float32)
    W2 = np.asarray(W2, np.float32)
    b2v = np.asarray(b2, np.float32)

    # table rows: [x_in | in_features | zeros]; row 0 of each half = zeros
    rows = np.zeros((TOK, 128), dtype=ml_dtypes.bfloat16)
    rows[1:N // 2 + 1, 0:C_IN] = x_in[:HALF].astype(ml_dtypes.bfloat16)
    rows[1:N // 2 + 1, C_IN:64] = in_features[:HALF].astype(ml_dtypes.bfloat16)
    tbl_lo = rows.reshape(RANKS, 128, 128).transpose(1, 0, 2).reshape(128, RANKS * 128).copy()
    rows[:] = 0
    rows[1:N // 2 + 1, 0:C_IN] = x_in[HALF:].astype(ml_dtypes.bfloat16)
    rows[1:N // 2 + 1, C_IN:64] = in_features[HALF:].astype(ml_dtypes.bfloat16)
    tbl_hi = rows.reshape(RANKS, 128, 128).transpose(1, 0, 2).reshape(128, RANKS * 128).copy()

    wx = np.zeros((64, HID), dtype=ml_dtypes.bfloat16)
    wx[0:C_IN] = W1[0:C_IN].astype(ml_dtypes.bfloat16)
    w1b = W1[C_IN:].astype(ml_dtypes.bfloat16)
    w2aug = np.zeros((HID + 1, C_OUT), dtype=ml_dtypes.bfloat16)
    w2aug[0:HID] = (W2 / DEG).astype(ml_dtypes.bfloat16)
    w2aug[HID] = (b2v / DEG).astype(ml_dtypes.bfloat16)
    b1c = b1v.reshape(HID, 1).copy()

    in_maps = []
    for c in range(NCORES):
        ilo, ihi, xo = _prep_core(
            idx[c * E_PER_CORE:(c + 1) * E_PER_CORE],
            x_out[c * SEG_PER_CORE:(c + 1) * SEG_PER_CORE])
        in_maps.append({
            "tbl_lo": tbl_lo, "tbl_hi": tbl_hi,
            "idx_lo": ilo, "idx_hi": ihi, "xo": xo,
            "wx": wx, "w1b": w1b, "w2": w2aug, "b1": b1c,
        })

    return in_maps


def kernel(**inputs):
    in_maps = prep_in_maps(**inputs)
    global _LAST_IN_MAPS
    _LAST_IN_MAPS = in_maps
    nc = build_nc()
    res = bass_utils.run_bass_kernel_spmd(nc, in_maps, list(range(NCORES))).results
    out = np.empty((M, C_OUT), np.float32)
    for c in range(NCORES):
        out[c * SEG_PER_CORE:(c + 1) * SEG_PER_CORE] = \
            res[c]["out"][:, :SEG_PER_CORE].T
    return out

